# revision 1
# baseline (speedup 1.0000x reference)
"""AttentionPooling (segment softmax-mean) Trainium2 kernel.

pooled[g] = mean over graph g of softmax_g(score)-weighted x rows, where
score_i = tanh(x_i @ w1 + b1) @ w2 + b2 and graph ids (batch) are sorted.

Strategy: 8 cores, graphs split contiguously and node-balanced. One pass
over x per core with unnormalized e_i = exp(score_i) (scores are O(1), no
overflow); per-graph S_g = sum e_i x_i and denom_g = sum e_i accumulate on
device; pooled = S_g / (denom_g * count_g) with host-known counts.

Per 512-node batch on device:
  DMA x -> PE transpose x (fp32, exact) -> ACT copy to f32r -> PE MLP
  (f32r) -> ACT tanh -> PE scores row (f32r) -> ACT exp -> e-row exported
  to DRAM (doubles as the bounce that re-loads it as per-partition columns
  via GPSIMD SWDGE) -> DVE weighted x -> PE transpose weighted (fp32,
  exact) -> DVE segment reduces -> SBUF accumulators. Denominators
  (sum of e per graph, ~0.001% of FLOPs) and the final per-graph scalar
  normalization are applied host-side from the exported e.

The Bass program is JIT-specialized per call: graph-slice boundaries from
the actual (sorted) batch vector are baked in as compile-time constants,
so each core gets its own program, built and compiled in parallel.
"""
import numpy as np

N_CORES = 8
D = 256
H = 128
NB = 512           # nodes per device batch
NCH = NB // 128    # 128-node chunks per batch


def _plan_shards(batch, num_graphs):
    counts = np.bincount(batch, minlength=num_graphs).astype(np.int64)
    starts = np.concatenate([[0], np.cumsum(counts)])  # [B+1]
    n = int(starts[-1])
    cuts = [0]
    for c in range(1, N_CORES):
        target = n * c // N_CORES
        g = int(np.searchsorted(starts, target, side="left"))
        g = max(cuts[-1] + 1, min(g, num_graphs - (N_CORES - c)))
        cuts.append(g)
    cuts.append(num_graphs)
    shards = []
    for c in range(N_CORES):
        g0, g1 = cuts[c], cuts[c + 1]
        n0, n1 = int(starts[g0]), int(starts[g1])
        shards.append(dict(g0=g0, g1=g1, n0=n0, n1=n1,
                           counts=counts[g0:g1],
                           gstarts=starts[g0:g1 + 1] - n0))
    return shards


def _slices_for_shard(sh):
    """Per batch: list of (a, b, piece, g_local) free-axis slices. piece is 0
    for the first piece of a graph, 1 for its continuation in a later batch."""
    nodes = sh["n1"] - sh["n0"]
    nb = (nodes + NB - 1) // NB
    npad = nb * NB
    G = sh["g1"] - sh["g0"]
    gstarts = sh["gstarts"]
    per_batch = []
    for b in range(nb):
        lo, hi = b * NB, min((b + 1) * NB, nodes)
        g = max(0, int(np.searchsorted(gstarts, lo, side="right")) - 1)
        sl = []
        while g < G and int(gstarts[g]) < hi:
            s, e = max(int(gstarts[g]), lo), min(int(gstarts[g + 1]), hi)
            if e > s:
                piece = 0 if s == int(gstarts[g]) else 1
                sl.append((s - lo, e - lo, piece, g))
            g += 1
        per_batch.append(sl)
    return nb, npad, G, per_batch


def _build_core_program(sh, b2f):
    import concourse.bass as bass
    import concourse.bacc as bacc
    import concourse.mybir as mybir
    import concourse.tile as tile

    nb, npad, G, per_batch = _slices_for_shard(sh)
    Gp = G + 1            # +1 trash column for the padding slice
    Gp2 = ((Gp + 127) // 128) * 128
    f32, f32r = mybir.dt.float32, mybir.dt.float32r
    AF = mybir.ActivationFunctionType

    nc = bacc.Bacc("TRN2", target_bir_lowering=False, debug=False)
    xs = nc.declare_dram_parameter("xs", [npad, D], f32, isOutput=False)
    w1_in = nc.declare_dram_parameter("w1", [D, H], f32, isOutput=False)
    b1_in = nc.declare_dram_parameter("b1", [H, 1], f32, isOutput=False)
    w2_in = nc.declare_dram_parameter("w2", [H, 1], f32, isOutput=False)
    ident_in = nc.declare_dram_parameter("ident", [128, 128], f32, isOutput=False)
    out_p = nc.declare_dram_parameter("pooled", [G, D], f32, isOutput=True)
    e_out = nc.declare_dram_parameter("e", [nb, NB], f32, isOutput=True)

    with tile.TileContext(nc) as tc:
        with tc.tile_pool(name="const", bufs=1) as const, \
             tc.tile_pool(name="acc", bufs=1) as accp, \
             tc.tile_pool(name="xp", bufs=10) as xp, \
             tc.tile_pool(name="xtp", bufs=8) as xtp, \
             tc.tile_pool(name="hp", bufs=6) as hp, \
             tc.tile_pool(name="ep", bufs=12) as ep, \
             tc.tile_pool(name="wp", bufs=6) as wp, \
             tc.tile_pool(name="fin", bufs=1) as fin, \
             tc.tile_pool(name="ps_xt", bufs=3, space="PSUM") as ps_xt, \
             tc.tile_pool(name="ps_h", bufs=3, space="PSUM") as ps_h, \
             tc.tile_pool(name="ps_wt", bufs=2, space="PSUM") as ps_wt:

            # ---- constants ----
            ident = const.tile([128, 128], f32, tag="ident")
            nc.sync.dma_start(out=ident, in_=ident_in[:, :])
            w1sb = const.tile([128, 2, H], f32, tag="w1sb")
            nc.sync.dma_start(out=w1sb,
                              in_=w1_in.rearrange("(c k) h -> k c h", c=2))
            b1col = const.tile([H, 1], f32, tag="b1col")
            nc.sync.dma_start(out=b1col, in_=b1_in[:, :])
            w2sb = const.tile([H, 1], f32, tag="w2sb")
            nc.sync.dma_start(out=w2sb, in_=w2_in[:, :])
            w1r = const.tile([128, 2, H], f32r, tag="w1r")
            nc.vector.tensor_copy(w1r, w1sb)
            w2r = const.tile([H, 1], f32r, tag="w2r")
            nc.vector.tensor_copy(w2r, w2sb)

            # ---- accumulators ----
            pacc = accp.tile([128, 2, 2, Gp], f32, tag="pacc")
            nc.vector.memset(pacc, 0.0)

            # ---- main loop ----
            for b in range(nb):
                x_t = xp.tile([128, NCH, D], f32, tag="x")
                nc.sync.dma_start(
                    out=x_t,
                    in_=xs[b * NB:(b + 1) * NB, :].rearrange(
                        "(c p) d -> p c d", p=128))

                xt_ps = [ps_xt.tile([128, NB], f32, tag="xt", name=f"xt{b}_{i}")
                         for i in range(2)]
                for c in range(NCH):
                    for f in range(2):
                        nc.tensor.transpose(
                            xt_ps[f][:, c * 128:(c + 1) * 128],
                            x_t[:, c, f * 128:(f + 1) * 128], ident)

                xt_hi = [xtp.tile([128, NB], f32r, tag="xthi", name=f"xthi{b}_{i}")
                         for i in range(2)]
                for f in range(2):
                    nc.scalar.copy(out=xt_hi[f], in_=xt_ps[f])

                h_ps = ps_h.tile([H, NB], f32, tag="h")
                for f in range(2):
                    nc.tensor.matmul(h_ps, w1r[:, f, :], xt_hi[f],
                                     start=(f == 0), stop=(f == 1))
                h_sb = hp.tile([H, NB], f32r, tag="hsb")
                nc.scalar.activation(out=h_sb, in_=h_ps, func=AF.Tanh,
                                     bias=b1col, scale=1.0)

                s_ps = ps_h.tile([1, NB], f32, tag="h")
                nc.tensor.matmul(s_ps, w2r, h_sb, start=True, stop=True)

                e_row = ep.tile([1, NB], f32, tag="erow")
                nc.scalar.activation(out=e_row, in_=s_ps, func=AF.Exp,
                                     bias=b2f, scale=1.0)
                nc.gpsimd.dma_start(out=e_out[b:b + 1, :], in_=e_row)
                e_cols = ep.tile([128, NCH], f32, tag="ecols")
                nc.gpsimd.dma_start(
                    out=e_cols,
                    in_=e_out[b, :].rearrange("(c p) -> p c", p=128))

                wx_t = wp.tile([128, NCH, D], f32, tag="wx")
                e_b = bass.AP(tensor=e_cols.tensor, offset=e_cols.offset,
                              ap=[list(e_cols.ap[0]), list(e_cols.ap[1]), [0, D]])
                nc.vector.tensor_mul(out=wx_t, in0=x_t, in1=e_b)
                wt_ps = [ps_wt.tile([128, NB], f32, tag="wt", name=f"wt{b}_{i}")
                         for i in range(2)]
                for c in range(NCH):
                    for f in range(2):
                        nc.tensor.transpose(
                            wt_ps[f][:, c * 128:(c + 1) * 128],
                            wx_t[:, c, f * 128:(f + 1) * 128], ident)

                for (a, e, piece, g) in per_batch[b]:
                    for f in range(2):
                        nc.vector.reduce_sum(
                            out=pacc[:, f, piece, g:g + 1],
                            in_=wt_ps[f][:, a:e],
                            axis=mybir.AxisListType.X)

            # ---- finalization ----
            pooled = fin.tile([128, 2, Gp], f32, tag="pooled")
            nc.vector.tensor_add(out=pooled, in0=pacc[:, :, 0, :],
                                 in1=pacc[:, :, 1, :])

            for gc in range((G + 127) // 128):
                gw = min(128, G - gc * 128)
                o_sb = fin.tile([128, D], f32, tag="osb", name=f"osb{gc}")
                for f in range(2):
                    pt_ps = ps_wt.tile([128, NB], f32, tag="wt", name=f"pt{gc}_{f}")
                    nc.tensor.transpose(
                        pt_ps[:gw, 0:128],
                        pooled[:, f, gc * 128:gc * 128 + gw], ident)
                    nc.vector.tensor_copy(
                        o_sb[:gw, f * 128:(f + 1) * 128], pt_ps[:gw, 0:128])
                nc.sync.dma_start(
                    out=out_p[gc * 128:gc * 128 + gw, :],
                    in_=o_sb[:gw, :])

    nc.compile()
    return nc, nb, npad, G


def kernel(x, batch, num_graphs, w1, b1, w2, b2):
    from concourse.bass_utils import run_bass_kernel_spmd

    x = np.asarray(x, dtype=np.float32)
    batch = np.asarray(batch).astype(np.int64)
    B = int(num_graphs)
    w1 = np.asarray(w1, dtype=np.float32)
    b1 = np.asarray(b1, dtype=np.float32)
    w2 = np.asarray(w2, dtype=np.float32)
    b2f = float(np.asarray(b2, dtype=np.float32).reshape(-1)[0])

    shards = _plan_shards(batch, B)
    ident = np.eye(128, dtype=np.float32)
    out = np.zeros((B, D), dtype=np.float32)

    import concurrent.futures as cf

    def build(c):
        sh = shards[c]
        nc, nb, npad, G = _build_core_program(sh, b2f)
        xs = np.zeros((npad, D), dtype=np.float32)
        xs[: sh["n1"] - sh["n0"]] = x[sh["n0"]: sh["n1"]]
        in_map = {"xs": xs, "w1": w1, "b1": b1.reshape(H, 1),
                  "w2": w2.reshape(H, 1), "ident": ident}
        return c, nc, in_map

    with cf.ThreadPoolExecutor(max_workers=8) as ex:
        built = list(ex.map(build, range(N_CORES)))

    for c, nc, in_map in built:
        res = run_bass_kernel_spmd(nc, [in_map], [0])
        sh = shards[c]
        nodes = sh["n1"] - sh["n0"]
        pooled = res.results[0]["pooled"].astype(np.float64)
        e_lin = res.results[0]["e"].reshape(-1)[:nodes].astype(np.float64)
        gstarts = sh["gstarts"]
        denom = np.add.reduceat(e_lin, gstarts[:-1]) if nodes else None
        # reduceat quirk: empty segments copy the element; recompute safely
        seg_len = np.diff(gstarts)
        if (seg_len == 0).any():
            denom = np.where(seg_len == 0, 0.0, denom)
        scale = denom * np.maximum(sh["counts"], 1.0)
        scale = np.where(seg_len == 0, 1.0, scale)
        pooled /= scale[:, None]
        pooled[seg_len == 0] = 0.0
        out[sh["g0"]: sh["g1"]] = pooled.astype(np.float32)
    return out



# revision 10
# speedup vs baseline: 1.7079x; 1.7079x over previous
"""AttentionPooling (segment softmax-mean) Trainium2 kernel.

pooled[g] = mean over graph g of softmax_g(score)-weighted x rows, where
score_i = tanh(x_i @ w1 + b1) @ w2 + b2 and graph ids (batch) are sorted.

Strategy: 8 cores, graphs split contiguously and node-balanced. One pass
over x per core with unnormalized e_i = exp(score_i) (scores are O(1), no
overflow). Host supplies x in TWO bf16 layouts (halving HBM traffic and
making every DMA line >=4KB):
  xT  [nsb, 128, 2, 2048]  feature-partitioned (for the MLP matmul)
  xn  [nsb, 128, 16, 256]  node-partitioned    (for the weighted pool)

Per 512-node batch on device (no PE transposes, no DVE reduces):
  PE h-matmul (w1 stationary, xT moving) -> ACT tanh -> PE score matmul
  with h as STATIONARY and w2 moving, so scores land node-partitioned
  [128,4] -> ACT exp -> DVE builds a [128 nodes, 128 graphs] e-weighted
  one-hot stationary -> PE matmul vs natural-x moving accumulates
  pooled[g, d] directly into a persistent PSUM tile across all batches.
e is kept in a resident SBUF strip and exported once; denominators and
the final per-graph scalar normalization are applied host-side.

The Bass program is JIT-specialized per call: graph-slice boundaries from
the actual (sorted) batch vector are baked in as compile-time constants,
so each core gets its own program, built and compiled in parallel.
"""
import numpy as np

N_CORES = 8
D = 256
H = 128
NB = 512            # nodes per compute batch
NCH = NB // 128     # 128-node chunks per batch
SB = 2048           # nodes per DMA super-batch
BPS = SB // NB      # batches per super-batch


def _plan_shards(batch, num_graphs):
    counts = np.bincount(batch, minlength=num_graphs).astype(np.int64)
    starts = np.concatenate([[0], np.cumsum(counts)])  # [B+1]
    n = int(starts[-1])
    cuts = [0]
    for c in range(1, N_CORES):
        target = n * c // N_CORES
        g = int(np.searchsorted(starts, target, side="left"))
        g = max(cuts[-1] + 1, min(g, num_graphs - (N_CORES - c)))
        cuts.append(g)
    cuts.append(num_graphs)
    shards = []
    for c in range(N_CORES):
        g0, g1 = cuts[c], cuts[c + 1]
        n0, n1 = int(starts[g0]), int(starts[g1])
        shards.append(dict(g0=g0, g1=g1, n0=n0, n1=n1,
                           counts=counts[g0:g1],
                           gstarts=starts[g0:g1 + 1] - n0))
    return shards


def _plan_batches(sh):
    """Per batch: {tile t: [(chunk, a, b, gcol), ...]} partition-spans of
    each local graph within each 128-node chunk, grouped by 128-graph
    PSUM tile."""
    nodes = sh["n1"] - sh["n0"]
    nb = (nodes + NB - 1) // NB
    nsb = (nodes + SB - 1) // SB
    G = sh["g1"] - sh["g0"]
    gstarts = sh["gstarts"]
    plans = []
    for b in range(nb):
        lo = b * NB
        groups = {}
        g = max(0, int(np.searchsorted(gstarts, lo, side="right")) - 1)
        for c in range(NCH):
            clo, chi = lo + c * 128, min(lo + (c + 1) * 128, nodes)
            if clo >= chi:
                break
            while g < G and int(gstarts[g]) < chi:
                s, e = max(int(gstarts[g]), clo), min(int(gstarts[g + 1]), chi)
                if e > s:
                    t = g // 128
                    groups.setdefault(t, []).append(
                        (c, s - clo, e - clo, g - t * 128))
                if int(gstarts[g + 1]) <= chi:
                    g += 1
                else:
                    break
        plans.append(groups)
    return nb, nsb, G, plans


def _build_core_program(sh, b2f):
    import concourse.bacc as bacc
    import concourse.mybir as mybir
    import concourse.tile as tile

    nb, nsb, G, plans = _plan_batches(sh)
    npad = nsb * SB
    ntiles = (G + 127) // 128
    assert ntiles <= 4
    f32, bf16 = mybir.dt.float32, mybir.dt.bfloat16
    AF = mybir.ActivationFunctionType

    # per-tile chunk-matmul counts, to place start/stop flags
    mm_total = [0] * ntiles
    nspan = 0
    for groups in plans:
        for t, sp in groups.items():
            mm_total[t] += len({c for (c, a, e, gc) in sp})
            nspan += len(sp)
    mm_seen = [0] * ntiles
    nspan_p = max(nspan, 1)

    nc = bacc.Bacc("TRN2", target_bir_lowering=False, debug=False)
    xT = nc.declare_dram_parameter("xT", [nsb, 128, 2, SB], bf16, isOutput=False)
    xn = nc.declare_dram_parameter("xn", [nsb, 128, SB // 128, D], bf16,
                                   isOutput=False)
    w1_in = nc.declare_dram_parameter("w1", [D, H], bf16, isOutput=False)
    b1_in = nc.declare_dram_parameter("b1", [H, 1], f32, isOutput=False)
    w2_in = nc.declare_dram_parameter("w2", [H, 1], bf16, isOutput=False)
    msk_in = nc.declare_dram_parameter("msk", [128, nspan_p], bf16,
                                       isOutput=False)
    out_p = nc.declare_dram_parameter("pooled", [G, D], f32, isOutput=True)
    e_out = nc.declare_dram_parameter("e", [128, NCH * nb], bf16, isOutput=True)

    with tile.TileContext(nc) as tc:
        with tc.tile_pool(name="const", bufs=1) as const, \
             tc.tile_pool(name="xtp", bufs=2) as xtp, \
             tc.tile_pool(name="xnp", bufs=2) as xnp, \
             tc.tile_pool(name="hp", bufs=3) as hp, \
             tc.tile_pool(name="ep", bufs=4) as ep, \
             tc.tile_pool(name="fin", bufs=1) as fin, \
             tc.tile_pool(name="ps_h", bufs=2, space="PSUM") as ps_h, \
             tc.tile_pool(name="ps_s", bufs=2, space="PSUM") as ps_s, \
             tc.tile_pool(name="ps_p", bufs=1, space="PSUM") as ps_p:

            # ---- constants ----
            w1sb = const.tile([128, 2, H], bf16, tag="w1sb")
            nc.sync.dma_start(out=w1sb,
                              in_=w1_in.rearrange("(f p) h -> p f h", f=2))
            b1col = const.tile([H, 1], f32, tag="b1col")
            nc.sync.dma_start(out=b1col, in_=b1_in[:, :])
            w2sb = const.tile([H, 1], bf16, tag="w2sb")
            nc.sync.dma_start(out=w2sb, in_=w2_in[:, :])
            msk = const.tile([128, nspan_p], bf16, tag="msk")
            nc.sync.dma_start(out=msk, in_=msk_in[:, :])

            # resident e strip: col b*NCH+c holds e for nodes [b*512+c*128+p]
            estore = const.tile([128, NCH * nb], bf16, tag="estore")

            # persistent pooled accumulators [graph, D] per 128-graph tile
            pp = [ps_p.tile([128, D], f32, tag="pp", name=f"pp{t}")
                  for t in range(ntiles)]

            si = 0
            for sb_i in range(nsb):
                xt_t = xtp.tile([128, 2, SB], bf16, tag="xt")
                nc.sync.dma_start(out=xt_t, in_=xT[sb_i])
                xn_t = xnp.tile([128, SB // 128, D], bf16, tag="xn")
                nc.scalar.dma_start(out=xn_t, in_=xn[sb_i])

                for bl in range(BPS):
                    b = sb_i * BPS + bl
                    if b >= nb:
                        break
                    groups = plans[b]

                    # h = tanh(x @ w1 + b1), feature-major [H, NB]
                    h_ps = ps_h.tile([H, NB], f32, tag="h")
                    for f in range(2):
                        nc.tensor.matmul(
                            h_ps, w1sb[:, f, :],
                            xt_t[:, f, bl * NB:(bl + 1) * NB],
                            start=(f == 0), stop=(f == 1))
                    h_sb = hp.tile([H, NB], bf16, tag="hsb")
                    nc.scalar.activation(out=h_sb, in_=h_ps, func=AF.Tanh,
                                         bias=b1col, scale=1.0)

                    # scores node-partitioned: lhsT = h chunk, rhs = w2
                    s_ps = ps_s.tile([128, NCH], f32, tag="s")
                    for c in range(NCH):
                        nc.tensor.matmul(
                            s_ps[:, c:c + 1],
                            h_sb[:, c * 128:(c + 1) * 128], w2sb,
                            start=True, stop=True)
                    nc.scalar.activation(
                        out=estore[:, b * NCH:(b + 1) * NCH], in_=s_ps,
                        func=AF.Exp, bias=b2f, scale=1.0)

                    # e-weighted one-hot stationary, pooled matmul
                    for t, sp in sorted(groups.items()):
                        eoh = ep.tile([128, NCH, 128], bf16, tag="eoh",
                                      name=f"eoh{b}_{t}")
                        nc.vector.memset(eoh, 0.0)
                        for (c, a, e, gc) in sp:
                            nc.vector.tensor_mul(
                                out=eoh[:, c, gc:gc + 1],
                                in0=estore[:, b * NCH + c:b * NCH + c + 1],
                                in1=msk[:, si:si + 1])
                            si += 1
                        for c in sorted({c for (c, a, e, gc) in sp}):
                            mm_seen[t] += 1
                            nc.tensor.matmul(
                                pp[t], eoh[:, c, :],
                                xn_t[:, bl * NCH + c, :],
                                start=(mm_seen[t] == 1),
                                stop=(mm_seen[t] == mm_total[t]))

            # ---- finalization ----
            for t in range(ntiles):
                if mm_total[t] == 0:
                    continue
                gw = min(128, G - t * 128)
                o_sb = fin.tile([128, D], f32, tag="osb", name=f"osb{t}")
                nc.vector.tensor_copy(o_sb, pp[t])
                nc.sync.dma_start(out=out_p[t * 128:t * 128 + gw, :],
                                  in_=o_sb[:gw, :])
            nc.sync.dma_start(out=e_out[:, :], in_=estore)

    nc.compile()
    return nc, nb, nsb, G


def _core_in_map(sh, x, w1, b1, w2):
    import ml_dtypes
    bf16 = ml_dtypes.bfloat16
    nodes = sh["n1"] - sh["n0"]
    nsb = (nodes + SB - 1) // SB
    npad = nsb * SB
    xp = np.zeros((npad, D), dtype=np.float32)
    xp[:nodes] = x[sh["n0"]:sh["n1"]]
    xb = xp.astype(bf16)
    # xT[s, p, f, n] = x[s*SB + n, f*128 + p]
    xT = np.ascontiguousarray(
        xb.reshape(nsb, SB, 2, 128).transpose(0, 3, 2, 1))
    # xn[s, p, c, d] = x[s*SB + c*128 + p, d]
    xn = np.ascontiguousarray(
        xb.reshape(nsb, SB // 128, 128, D).transpose(0, 2, 1, 3))
    # span masks, in program emission order (b, t asc, span order)
    nb2, nsb2, G, plans = _plan_batches(sh)
    spans = [s for groups in plans
             for t, sp in sorted(groups.items()) for s in sp]
    mskf = np.zeros((128, max(len(spans), 1)), np.float32)
    for i, (c, a, e, gc) in enumerate(spans):
        mskf[a:e, i] = 1.0
    return {"xT": xT, "xn": xn,
            "w1": np.asarray(w1, np.float32).astype(bf16),
            "b1": np.asarray(b1, np.float32).reshape(H, 1),
            "w2": np.asarray(w2, np.float32).astype(bf16).reshape(H, 1),
            "msk": mskf.astype(bf16)}


def _finalize(sh, res, out):
    """Host: divide pooled sums by (sum_g e) * count_g."""
    nodes = sh["n1"] - sh["n0"]
    nb = (nodes + NB - 1) // NB
    pooled = res["pooled"].astype(np.float64)
    e_lin = res["e"].astype(np.float64).T.reshape(-1)[:nodes]
    gstarts = sh["gstarts"]
    seg_len = np.diff(gstarts)
    denom = np.add.reduceat(e_lin, gstarts[:-1]) if nodes else None
    if (seg_len == 0).any():
        denom = np.where(seg_len == 0, 0.0, denom)
    scale = denom * np.maximum(sh["counts"], 1.0)
    scale = np.where(seg_len == 0, 1.0, scale)
    pooled /= scale[:, None]
    pooled[seg_len == 0] = 0.0
    out[sh["g0"]:sh["g1"]] = pooled.astype(np.float32)


def kernel(x, batch, num_graphs, w1, b1, w2, b2):
    from concourse.bass_utils import run_bass_kernel_spmd

    x = np.asarray(x, dtype=np.float32)
    batch = np.asarray(batch).astype(np.int64)
    B = int(num_graphs)
    b2f = float(np.asarray(b2, dtype=np.float32).reshape(-1)[0])

    shards = _plan_shards(batch, B)
    out = np.zeros((B, D), dtype=np.float32)

    import concurrent.futures as cf

    def build(c):
        sh = shards[c]
        nc, nb, nsb, G = _build_core_program(sh, b2f)
        in_map = _core_in_map(sh, x, w1, b1, w2)
        return c, nc, in_map

    with cf.ThreadPoolExecutor(max_workers=8) as ex:
        built = list(ex.map(build, range(N_CORES)))

    for c, nc, in_map in built:
        res = run_bass_kernel_spmd(nc, [in_map], [0])
        _finalize(shards[c], res.results[0], out)
    return out
